# revision 1
# baseline (speedup 1.0000x reference)
"""GCN (2-layer GCNConv + log_softmax) on 8 Trainium2 NeuronCores.

Strategy:
  - Nodes sharded contiguously across 8 cores (12500 each). Layer matmuls
    (x@W1, @W2-delta) run on PE per core.
  - Per-layer scaled features g = dinv * (h@W) are all-gathered into a
    per-core DRAM table (rows padded to 256B so dma_gather can stride).
  - Edge aggregation: per-edge dma_gather (bf16, 32B rows) over dst-sorted,
    degree-class-grouped edge streams; DVE tensor_reduce does the fixed-
    length segment sums; dma_scatter_add (CCE) merges per-bin partial acc
    into a DRAM accumulator initialized with the self-loop term.
  - log_softmax over 2 classes computed as softplus of the logit delta.

All graph preprocessing (sharding, sorting, degree classes, int16 index
streams) happens on host in numpy; all FP math runs on device.
"""

import os
import numpy as np
import ml_dtypes

# ----------------------------------------------------------------------------
# constants
# ----------------------------------------------------------------------------
NCORES = 8
N_FEAT = 256
HID = 16
WIN = int(os.environ.get("GCN_WIN", "32767"))   # int16 index window (rows/bin)
MAX_CLASS = 64       # segments longer than this are split
CHUNK_SLOTS = 64     # msg slots per gather call (64*128 = 8192 idx max)

_bf16 = ml_dtypes.bfloat16

_EXEC_TIME_NS = [None]   # filled when GCN_TRACE=1
_LAST_RES = [None]


def _install_patches():
    import json as _json
    import types
    import concourse.tile as tile
    import concourse.mybir as mybir
    from concourse.vector_clock import ScopedClock

    # --- final drain: one wait per nop (walrus rejects multi-wait insts) ---
    def _drain_and_barrier_split(self, tick_clock, wait_clock):
        nc = self.nc
        anchor = nc.sync.nop(hint="drain_wait_anchor", nofuse=True)
        wait_clock.add_sem_waits(
            anchor.ins, ScopedClock({None: tick_clock.global_clock})
        )
        waits = list(anchor.ins.sync_info.on_wait)
        anchor.ins.sync_info.on_wait = waits[:1]
        for w in waits[1:]:
            nop_inst = nc.sync.nop(hint="drain_wait_split", nofuse=True)
            nop_inst.ins.sync_info = mybir.SyncInfo(on_wait=[w], on_update=[])
        nc.sync.drain()
        nc.all_engine_barrier()
        assert self.sems is not None
        popped = nc._tile_sem_poison_stack.pop()
        assert popped is self._sem_poison
        nc.clear_and_free_semaphores(list(self.sems.allocated().values()))
        nc.all_engine_barrier()

    if getattr(tile.TileContext, "_gcn_patched", False):
        return
    tile.TileContext._drain_and_barrier = _drain_and_barrier_split
    tile.TileContext._gcn_patched = True

    # --- BIR post-pass: hoist excess sync waits onto EventSemaphore nops ---
    ctr = [0]

    def _split_bir_waits(bir_json: bytes) -> bytes:
        d = _json.loads(bir_json)
        changed = False
        for fn in d.get("functions", []):
            for blk in fn.get("blocks", []):
                new_insts = []
                for ins in blk.get("instructions", []):
                    si = ins.get("sync_info")
                    waits = (si or {}).get("on_wait") or []
                    if len(waits) > 1:
                        for w in waits[1:]:
                            ctr[0] += 1
                            new_insts.append({
                                "debug": ins.get("debug", 0),
                                "engine": ins["engine"],
                                "ins": [], "outs": [],
                                "name": f"waitsplit-{ctr[0]}",
                                "opcode": "EventSemaphore",
                                "sync_info": {"on_update": [],
                                              "on_wait": [w]},
                            })
                        si["on_wait"] = waits[:1]
                        changed = True
                    new_insts.append(ins)
                blk["instructions"] = new_insts
        return _json.dumps(d).encode() if changed else bir_json

    import concourse.bass_utils as bass_utils
    import concourse.bass2jax as bass2jax

    orig_compile = bass_utils.compile_bir_kernel

    def compile_split(bir_json, tmpdir, neff_name="file.neff"):
        return orig_compile(_split_bir_waits(bir_json), tmpdir, neff_name)

    bass_utils.compile_bir_kernel = compile_split
    bass2jax.compile_bir_kernel = compile_split
    bass_utils.upload_artifacts = lambda tmpdir: f"file://{tmpdir}"

    # --- NTFF profiling hook (image's antenv lacks axon_hooks) ---
    import sys
    if "antenv.axon_hooks" not in sys.modules:
        mod = types.ModuleType("antenv.axon_hooks")
        hook = [None]
        mod.set_axon_ntff_profile_hook = lambda h: hook.__setitem__(0, h)
        mod.get_axon_ntff_profile_hook = lambda: hook[0]
        sys.modules["antenv.axon_hooks"] = mod
        try:
            import antenv
            antenv.axon_hooks = mod
        except ImportError:
            pass
        try:
            from trn_agent_boot.trn_boot import _ntff_profile_via_ctypes
            so = "/opt/axon/libaxon_pjrt.so"
            if os.path.exists(so):
                mod.set_axon_ntff_profile_hook(_ntff_profile_via_ctypes(so))
        except Exception:
            pass


def _dma_gather_raw(gpsimd, out_ap, in_ap, idxs_ap, num_idxs, elem_size,
                    elem_step):
    """bass dma_gather without the (transpose-only) elem%256B assert."""
    import concourse.mybir as mybir
    from concourse import ap_utils
    from concourse._compat import exact_div
    from concourse.bass import MemorySpace, round_up_to_multiple

    self = gpsimd
    assert idxs_ap.dtype == mybir.dt.int16
    assert in_ap.space == MemorySpace.DRAM
    assert idxs_ap.space == MemorySpace.SBUF
    assert out_ap.space == MemorySpace.SBUF
    assert in_ap.dtype == out_ap.dtype
    assert ap_utils.ap_is_contiguous(out_ap.ap[1:])
    assert ap_utils.ap_is_contiguous(idxs_ap.ap[1:])
    assert in_ap.ap[-1][1] == out_ap.ap[-1][1] == elem_size
    assert out_ap.ap[0][1] * out_ap.ap[1][1] == round_up_to_multiple(num_idxs, 128)
    assert in_ap.ap[0][0] == elem_step
    stride_bytes_256 = exact_div(elem_step * mybir.dt.size(in_ap.dtype), 256)
    assert 0 < stride_bytes_256 < 256

    _in_ap = self.lower_ap_dma(in_ap, for_custom_bir_dma=True)
    _idxs_ap = self.lower_ap(idxs_ap)
    _out_ap = self.lower_ap(out_ap)
    return self.add_instruction(
        mybir.InstDMAGatherAnt(
            name=self.bass.get_next_instruction_name(),
            ins=[*_in_ap, _idxs_ap,
                 self.lower_val_access(self.to_reg(num_idxs))],
            outs=[_out_ap],
            transpose=False, num_idxs=num_idxs, elem_size=elem_size,
            stride_bytes_256=stride_bytes_256, gen_mode=0,
            single_packet=False, queue_num=0,
            sbuf_tokens_per_rank=0, sbuf_free_dim_per_rank=0,
            sbuf_free_dim_pad_per_rank=0, sbuf_byte_offset=0,
        )
    )


# ----------------------------------------------------------------------------
# host-side graph preprocessing
# ----------------------------------------------------------------------------
def _wrap16(stream):
    """Flat int16 stream -> [128, L/16] (idx j <-> partition j%16, col j//16,
    replicated across the 8 Q7 core groups)."""
    L = stream.shape[0]
    assert L % 16 == 0
    arr = stream.reshape(L // 16, 16).T.astype(np.int16)   # [16, cols]
    return np.tile(arr, (8, 1)).copy()


def _prep(edge_index, n_nodes):
    """Build per-core aggregation plan + index streams."""
    npc = n_nodes // NCORES
    npad = ((npc + 127) // 128) * 128
    tiles = npad // 128
    rps = npad + 1                       # rows per shard (+1 zero row)
    tbl_rows = NCORES * rps
    nbins = (tbl_rows + WIN - 1) // WIN

    src = np.asarray(edge_index[0], dtype=np.int64)
    dst = np.asarray(edge_index[1], dtype=np.int64)
    deg = (np.bincount(dst, minlength=n_nodes) + 1).astype(np.float32)

    # node -> (core, r_loc) ; r_loc = (l%128)*tiles + l//128 so that the
    # device's (partition, tile) DMA iteration order is row-sequential.
    def r_of(local):
        return (local % 128) * tiles + local // 128

    c_src = src // npc
    l_src = src - c_src * npc
    row_src = c_src * rps + r_of(l_src)
    bin_src = row_src // WIN
    loc_src = (row_src - bin_src * WIN).astype(np.int16)

    c_dst = dst // npc
    dst_r = r_of(dst - c_dst * npc)

    # one zero row per bin: shard-trailing rows c*rps + npad
    zrows = np.arange(NCORES) * rps + npad
    zloc = np.full(nbins, -1, dtype=np.int64)
    for zr in zrows:
        w = zr // WIN
        if zloc[w] < 0:
            zloc[w] = zr - w * WIN
    assert (zloc >= 0).all(), f"no zero row in some bin: {zloc}"

    trash = npad                          # acc trash row

    # ---- per (core, bin): segment structure ----
    # seg_data[c][w] = dict d -> (locs [n_d, d], dsts [n_d])
    seg_data = [[dict() for _ in range(nbins)] for _ in range(NCORES)]
    order = np.lexsort((loc_src, dst_r, bin_src, c_dst))
    cs, bs, ds_, ls = c_dst[order], bin_src[order], dst_r[order], loc_src[order]
    # boundaries of (core, bin, dst) groups
    key = (cs * nbins + bs) * npad + ds_
    starts = np.flatnonzero(np.r_[True, key[1:] != key[:-1]])
    counts = np.diff(np.r_[starts, key.shape[0]])
    g_core = cs[starts]
    g_bin = bs[starts]
    g_dst = ds_[starts]
    for c in range(NCORES):
        for w in range(nbins):
            m = (g_core == c) & (g_bin == w)
            if not m.any():
                continue
            st, ct, dr = starts[m], counts[m], g_dst[m]
            assert ct.max() <= MAX_CLASS, f"degree class {ct.max()} > {MAX_CLASS}"
            for d in np.unique(ct):
                mm = ct == d
                idx2 = st[mm][:, None] + np.arange(d)[None, :]
                seg_data[c][w][int(d)] = (ls[idx2], dr[mm])

    # ---- common (max-padded) structure per bin ----
    plan_bins = []
    gidx_all = [[] for _ in range(NCORES)]
    sidx_all = [[] for _ in range(NCORES)]
    for w in range(nbins):
        classes = sorted({d for c in range(NCORES) for d in seg_data[c][w]})
        S = {d: max((seg_data[c][w][d][0].shape[0] if d in seg_data[c][w]
                     else 0) for c in range(NCORES)) for d in classes}
        S = {d: (S[d] + 127) // 128 for d in classes}     # rounds of 128 segs
        # piece slot offsets per class
        poff, T = {}, 0
        for d in classes:
            poff[d] = T
            T += S[d]
        # gather calls: walk classes/rounds, cut at <= CHUNK_SLOTS slots
        calls = []
        cur = {"slots": 0, "reduces": []}
        for d in classes:
            s = 0
            while s < S[d]:
                if cur["slots"] + d > CHUNK_SLOTS:
                    calls.append(cur)
                    cur = {"slots": 0, "reduces": []}
                take = min((CHUNK_SLOTS - cur["slots"]) // d, S[d] - s)
                cur["reduces"].append(
                    dict(off=cur["slots"], S=take, d=d, poff=poff[d] + s))
                cur["slots"] += take * d
                s += take
        if cur["slots"]:
            calls.append(cur)
        # scatter calls: piece slots chunked
        scalls = []
        a = 0
        while a < T:
            b = min(a + 32, T)          # scatter_add caps out below 8192 idx
            scalls.append((a, b))
            a = b
        plan_bins.append(dict(classes=classes, S=S, poff=poff, T=T,
                              calls=calls, scalls=scalls,
                              gcols=sum(c["slots"] for c in calls) * 8,
                              scols=T * 8))

        # ---- per-core index streams ----
        for c in range(NCORES):
            gparts, sparts = [], []
            for d in classes:
                nrounds = S[d]
                locs, drs = seg_data[c][w].get(d, (np.zeros((0, d), np.int16),
                                                  np.zeros(0, np.int64)))
                n = locs.shape[0]
                N_d = nrounds * 128
                locs_p = np.full((N_d, d), zloc[w], dtype=np.int16)
                locs_p[:n] = locs
                dst_p = np.full(N_d, trash, dtype=np.int64)
                dst_p[:n] = drs
                # seg m -> (p=m%128, s=m//128); row ((s*d)+r)*128+p
                g = locs_p.reshape(nrounds, 128, d).transpose(0, 2, 1)
                gparts.append(g.reshape(-1))
                sparts.append(dst_p.reshape(nrounds, 128).reshape(-1))
            gidx_all[c].append(_wrap16(np.concatenate(gparts)
                                       if gparts else np.zeros(0, np.int16)))
            sidx_all[c].append(_wrap16(np.concatenate(sparts).astype(np.int16)
                                       if sparts else np.zeros(0, np.int16)))

    plan = dict(npc=npc, npad=npad, tiles=tiles, rps=rps,
                tbl_rows=tbl_rows, nbins=nbins, trash=trash,
                bins=plan_bins, zrows=zrows)
    return plan, gidx_all, sidx_all, deg


# ----------------------------------------------------------------------------
# bass kernel builder
# ----------------------------------------------------------------------------
def _build_bass(plan):
    import concourse.bass as bass
    import concourse.bacc as bacc
    import concourse.mybir as mybir
    import concourse.tile as tile

    npad, tiles, rps = plan["npad"], plan["tiles"], plan["rps"]
    tbl_rows, nbins = plan["tbl_rows"], plan["nbins"]
    acc_rows = npad + 1
    f32, bf16, i16 = mybir.dt.float32, mybir.dt.bfloat16, mybir.dt.int16
    AP = bass.AP

    nc = bacc.Bacc(None, target_bir_lowering=False)

    xT = nc.declare_dram_parameter("xT", [128, 2 * npad], bf16, isOutput=False)
    w1 = nc.declare_dram_parameter("w1", [128, 32], bf16, isOutput=False)
    degp = nc.declare_dram_parameter("degp", [128, tiles], f32, isOutput=False)
    b1v = nc.declare_dram_parameter("b1v", [128, 16], f32, isOutput=False)
    wd = nc.declare_dram_parameter("wd", [128, 16], f32, isOutput=False)
    bd = nc.declare_dram_parameter("bd", [128, 1], f32, isOutput=False)
    gidx_p = [nc.declare_dram_parameter(f"gidx{w}", [128, max(plan["bins"][w]["gcols"], 16)],
                                        i16, isOutput=False) for w in range(nbins)]
    sidx_p = [nc.declare_dram_parameter(f"sidx{w}", [128, max(plan["bins"][w]["scols"], 16)],
                                        i16, isOutput=False) for w in range(nbins)]
    out_p = nc.declare_dram_parameter("out", [npad, 2], f32, isOutput=True)

    tables = [nc.dram_tensor(f"table{l}", [tbl_rows, 128], bf16) for l in (0, 1)]
    agins = [nc.dram_tensor(f"agin{l}", [rps, 128], bf16) for l in (0, 1)]
    accs = [nc.dram_tensor(f"acc{l}", [acc_rows, 64], f32) for l in (0, 1)]

    def view(ap, dims, extra_off=0):
        return AP(ap.tensor, ap.offset + extra_off, dims)

    with tile.TileContext(nc) as tc:
        with tc.tile_pool(name="sb", bufs=1) as P1, \
             tc.tile_pool(name="dbl", bufs=2) as P2, \
             tc.tile_pool(name="ps", bufs=2, space="PSUM") as PP:

            # --- constants in ---
            w1_t = P1.tile([128, 32], bf16)
            nc.sync.dma_start(out=w1_t[:], in_=w1[:])
            b1_t = P1.tile([128, 16], f32)
            nc.sync.dma_start(out=b1_t[:], in_=b1v[:])
            wd_t = P1.tile([128, 16], f32)
            nc.sync.dma_start(out=wd_t[:], in_=wd[:])
            bd_t = P1.tile([128, 1], f32)
            nc.sync.dma_start(out=bd_t[:], in_=bd[:])
            deg_t = P1.tile([128, tiles], f32)
            nc.sync.dma_start(out=deg_t[:], in_=degp[:])
            dinv = P1.tile([128, tiles], f32)
            nc.scalar.activation(dinv[:], deg_t[:],
                                 mybir.ActivationFunctionType.Ln)
            nc.scalar.activation(dinv[:], dinv[:],
                                 mybir.ActivationFunctionType.Exp,
                                 bias=0.0, scale=-0.5)

            zero_bf = P1.tile([1, 128], bf16)
            nc.vector.memset(zero_bf[:], 0.0)

            # --- x @ W1 -> g1 = dinv * hw ---
            g1 = P1.tile([128, tiles, 16], f32)
            xk = P1.tile([128, npad], bf16)
            for kc in (0, 1):
                nc.sync.dma_start(out=xk[:], in_=xT[:, kc * npad:(kc + 1) * npad])
                for t in range(tiles):
                    ps = PP.tile([128, 16], f32, tag="mm")
                    nc.tensor.matmul(out=ps[:],
                                     lhsT=xk[:, t * 128:(t + 1) * 128],
                                     rhs=w1_t[:, kc * 16:(kc + 1) * 16],
                                     start=True, stop=True)
                    if kc == 0:
                        nc.vector.tensor_copy(out=g1[:, t, :], in_=ps[:])
                    else:
                        nc.vector.tensor_add(out=g1[:, t, :], in0=g1[:, t, :],
                                             in1=ps[:])
            dinv_b = dinv[:, :].unsqueeze(-1).to_broadcast([128, tiles, 16])
            nc.vector.tensor_mul(out=g1[:, :, :], in0=g1[:, :, :], in1=dinv_b)

            stage = P1.tile([128, tiles, 128], bf16)
            nc.vector.memset(stage[:], 0.0)
            accst = P1.tile([128, tiles, 64], f32)
            nc.vector.memset(accst[:], 0.0)

            gsrc = g1
            for layer in (0, 1):
                table, agin, acc = tables[layer], agins[layer], accs[layer]
                binfo = plan["bins"]

                # table build: padded shard -> allgather
                nc.vector.tensor_copy(out=stage[:, :, :16], in_=gsrc[:, :, :])
                nc.sync.dma_start(out=agin[:npad, :], in_=stage[:, :, :])
                nc.sync.dma_start(out=agin[npad:npad + 1, :], in_=zero_bf[:])
                nc.gpsimd.collective_compute(
                    "AllGather", mybir.AluOpType.bypass,
                    replica_groups=[list(range(NCORES))],
                    ins=[agin[:, :].opt()], outs=[table[:, :].opt()])

                # acc init = self-loop term
                nc.vector.tensor_copy(out=accst[:, :, :16], in_=gsrc[:, :, :])
                nc.sync.dma_start(out=acc[:npad, :], in_=accst[:, :, :])

                # aggregation
                for w in range(nbins):
                    bi = binfo[w]
                    if bi["T"] == 0:
                        continue
                    gi = P2.tile([128, max(bi["gcols"], 16)], i16, tag="gi")
                    nc.sync.dma_start(out=gi[:], in_=gidx_p[w][:])
                    si = P2.tile([128, max(bi["scols"], 16)], i16, tag="si")
                    nc.sync.dma_start(out=si[:], in_=sidx_p[w][:])
                    piece = P2.tile([128, bi["T"], 16], f32, tag="piece")
                    win_lo = w * WIN
                    win_n = min(WIN, tbl_rows - win_lo)
                    tbl_win = table[win_lo:win_lo + win_n, :16]
                    col = 0
                    for call in bi["calls"]:
                        slots = call["slots"]
                        rows = slots * 128
                        msg = P2.tile([128, CHUNK_SLOTS, 16], bf16, tag="msg")
                        _dma_gather_raw(
                            nc.gpsimd,
                            out_ap=msg[:, :slots, :],
                            in_ap=tbl_win,
                            idxs_ap=gi[:, col:col + slots * 8],
                            num_idxs=rows, elem_size=16, elem_step=128)
                        col += slots * 8
                        base = msg[:, :, :]
                        for r in call["reduces"]:
                            rv = view(base,
                                      [base.ap[0],
                                       (r["d"] * 16, r["S"]),
                                       (1, 16),
                                       (16, r["d"])],
                                      extra_off=r["off"] * 16)
                            nc.vector.tensor_reduce(
                                out=piece[:, r["poff"]:r["poff"] + r["S"], :],
                                in_=rv, axis=mybir.AxisListType.X,
                                op=mybir.AluOpType.add)
                    for (a, b) in bi["scalls"]:
                        nc.gpsimd.dma_scatter_add(
                            out_ap=acc[:, :16],
                            in_ap=piece[:, a:b, :],
                            idxs_ap=si[:, a * 8:b * 8],
                            num_idxs=(b - a) * 128,
                            num_idxs_reg=(b - a) * 128,
                            elem_size=16, elem_step=64, single_packet=False)

                # epilogue: readback + pointwise
                rb = P1.tile([128, tiles, 64], f32, tag="rb")
                nc.sync.dma_start(out=rb[:, :, :], in_=acc[:npad, :])
                q = P1.tile([128, tiles, 16], f32, tag="q")
                nc.vector.tensor_mul(out=q[:, :, :], in0=rb[:, :, :16],
                                     in1=dinv_b)
                if layer == 0:
                    b1_b = b1_t[:, :].unsqueeze(1).to_broadcast([128, tiles, 16])
                    nc.vector.tensor_add(out=q[:, :, :], in0=q[:, :, :],
                                         in1=b1_b)
                    h = P1.tile([128, tiles, 16], f32, tag="scr16")
                    nc.vector.tensor_scalar(out=h[:, :, :], in0=q[:, :, :],
                                            scalar1=0.0, scalar2=None,
                                            op0=mybir.AluOpType.max)
                    g2 = P1.tile([128, tiles, 16], f32)
                    nc.vector.tensor_mul(out=g2[:, :, :], in0=h[:, :, :],
                                         in1=dinv_b)
                    gsrc = g2
                else:
                    wd_b = wd_t[:, :].unsqueeze(1).to_broadcast([128, tiles, 16])
                    tmp = P1.tile([128, tiles, 16], f32, tag="scr16")
                    nc.vector.tensor_mul(out=tmp[:, :, :], in0=q[:, :, :],
                                         in1=wd_b)
                    delta = P1.tile([128, tiles, 1], f32)
                    nc.vector.tensor_reduce(out=delta[:, :, :], in_=tmp[:, :, :],
                                            axis=mybir.AxisListType.X,
                                            op=mybir.AluOpType.add)
                    bd_b = bd_t[:, :].unsqueeze(1).to_broadcast([128, tiles, 1])
                    nc.vector.tensor_add(out=delta[:, :, :], in0=delta[:, :, :],
                                         in1=bd_b)
                    # softplus(d) = m + ln(exp(-m) + exp(d-m)), m = max(d, 0)
                    m_t = P1.tile([128, tiles, 1], f32)
                    nc.vector.tensor_scalar(out=m_t[:, :, :],
                                            in0=delta[:, :, :],
                                            scalar1=0.0, scalar2=None,
                                            op0=mybir.AluOpType.max)
                    e1 = P1.tile([128, tiles, 1], f32)
                    nc.vector.tensor_sub(out=e1[:, :, :], in0=delta[:, :, :],
                                         in1=m_t[:, :, :])
                    nc.scalar.activation(e1[:, :, :], e1[:, :, :],
                                         mybir.ActivationFunctionType.Exp)
                    e2 = P1.tile([128, tiles, 1], f32)
                    nc.scalar.activation(e2[:, :, :], m_t[:, :, :],
                                         mybir.ActivationFunctionType.Exp,
                                         bias=0.0, scale=-1.0)
                    sp = P1.tile([128, tiles, 1], f32)
                    nc.vector.tensor_add(out=sp[:, :, :], in0=e1[:, :, :],
                                         in1=e2[:, :, :])
                    nc.scalar.activation(sp[:, :, :], sp[:, :, :],
                                         mybir.ActivationFunctionType.Ln)
                    nc.vector.tensor_add(out=sp[:, :, :], in0=sp[:, :, :],
                                         in1=m_t[:, :, :])
                    outt = P1.tile([128, tiles, 2], f32)
                    nc.vector.tensor_scalar(out=outt[:, :, 0:1], in0=sp[:, :, :],
                                            scalar1=-1.0, scalar2=None,
                                            op0=mybir.AluOpType.mult)
                    nc.vector.tensor_sub(out=outt[:, :, 1:2], in0=delta[:, :, :],
                                         in1=sp[:, :, :])
                    nc.sync.dma_start(out=out_p[:, :], in_=outt[:, :, :])

    nc.finalize()
    return nc


# ----------------------------------------------------------------------------
# public entry
# ----------------------------------------------------------------------------
_CACHE = {}


def kernel(x, edge_index, W1, b1, W2, b2):
    _install_patches()
    from concourse.bass_utils import run_bass_kernel_spmd

    n = x.shape[0]
    plan, gidx_all, sidx_all, deg = _prep(edge_index, n)
    npc, npad, tiles = plan["npc"], plan["npad"], plan["tiles"]

    key = (n, tuple(tuple((b["gcols"], b["scols"],
                           tuple(c["slots"] for c in b["calls"]))
                          for b in plan["bins"])))
    if key not in _CACHE:
        _CACHE.clear()
        _CACHE[key] = _build_bass(plan)
    nc = _CACHE[key]

    wdiff = (W2[:, 1] - W2[:, 0]).astype(np.float32)
    bdiff = np.float32(b2[1] - b2[0])

    in_maps = []
    for c in range(NCORES):
        xc = np.zeros((npad, N_FEAT), np.float32)
        xc[:npc] = x[c * npc:(c + 1) * npc]
        # xT[p, kc*npad + j] = xc[j, kc*128+p]
        xT = np.ascontiguousarray(
            xc.T.reshape(2, 128, npad).transpose(1, 0, 2).reshape(128, 2 * npad)
        ).astype(_bf16)
        degc = np.ones(npad, np.float32)
        degc[:npc] = deg[c * npc:(c + 1) * npc]
        degp = np.ascontiguousarray(degc.reshape(tiles, 128).T)
        w1p = np.ascontiguousarray(
            W1.astype(np.float32).reshape(2, 128, 16).transpose(1, 0, 2)
            .reshape(128, 32)).astype(_bf16)
        m = dict(xT=xT, w1=w1p, degp=degp,
                 b1v=np.tile(b1.reshape(1, 16).astype(np.float32), (128, 1)),
                 wd=np.tile(wdiff.reshape(1, 16), (128, 1)),
                 bd=np.full((128, 1), bdiff, np.float32))
        for w in range(plan["nbins"]):
            g = gidx_all[c][w]
            s = sidx_all[c][w]
            if g.shape[1] == 0:
                g = np.zeros((128, 16), np.int16)
            if s.shape[1] == 0:
                s = np.zeros((128, 16), np.int16)
            m[f"gidx{w}"] = np.ascontiguousarray(g)
            m[f"sidx{w}"] = np.ascontiguousarray(s)
        in_maps.append(m)

    trace = bool(int(os.environ.get("GCN_TRACE", "0")))
    res = run_bass_kernel_spmd(nc, in_maps, core_ids=list(range(NCORES)),
                               trace=trace)
    _EXEC_TIME_NS[0] = res.exec_time_ns
    _LAST_RES[0] = res

    out = np.empty((n, 2), np.float32)
    l = np.arange(npc)
    r_loc = (l % 128) * tiles + l // 128
    for c in range(NCORES):
        out[c * npc:(c + 1) * npc] = res.results[c]["out"][r_loc]
    return out



# revision 2
# speedup vs baseline: 2.5416x; 2.5416x over previous
"""GCN (2-layer GCNConv + log_softmax) on 8 Trainium2 NeuronCores — v2.

Architecture (per core, per layer):
  - Feature-transposed tables: [16 feat, nodes] bf16, replicated per Q7
    core group.  Aggregation via gpsimd ap_gather (8 concurrent per-group
    edge streams, one index per edge, d=2 node-pairs), DVE mask-multiply
    (mask carries the full sym-norm and parity selection), DVE segment
    reduce over a rank-capacity grid shared by all 64 (core,group)
    streams, gpsimd ap_gather alignment across the 2 table windows.
  - Edges partitioned by dst into 8 groups per core (one per Q7 core);
    layer matmuls on PE produce transposed shards directly; AllGather
    exchanges transposed shards; W2 delta folded into the layer-2 table.
"""

import os
import numpy as np
import ml_dtypes

NCORES = 8
NPC = 12500            # nodes per core
SHARD = 12544          # padded shard width (= 98*128)
GS = 1568              # group stride in layer-2 column space
NSEG = 1563            # max segments per group (4 groups of 1563, 4 of 1562)
WINN = 50176           # nodes per window (4 shards)
NW = 2                 # windows
CHUNK = 3840           # ap_gather slots per chunk
N_FEAT = 256
HID = 16

_bf16 = ml_dtypes.bfloat16
_EXEC_TIME_NS = [None]
_LAST_RES = [None]

_GB = np.r_[0, np.cumsum([1563] * 4 + [1562] * 4)]   # group row boundaries


def _install_patches():
    import json as _json
    import types
    import concourse.tile as tile
    import concourse.mybir as mybir
    from concourse.vector_clock import ScopedClock

    def _drain_and_barrier_split(self, tick_clock, wait_clock):
        nc = self.nc
        anchor = nc.sync.nop(hint="drain_wait_anchor", nofuse=True)
        wait_clock.add_sem_waits(
            anchor.ins, ScopedClock({None: tick_clock.global_clock})
        )
        waits = list(anchor.ins.sync_info.on_wait)
        anchor.ins.sync_info.on_wait = waits[:1]
        for w in waits[1:]:
            nop_inst = nc.sync.nop(hint="drain_wait_split", nofuse=True)
            nop_inst.ins.sync_info = mybir.SyncInfo(on_wait=[w], on_update=[])
        nc.sync.drain()
        nc.all_engine_barrier()
        assert self.sems is not None
        popped = nc._tile_sem_poison_stack.pop()
        assert popped is self._sem_poison
        nc.clear_and_free_semaphores(list(self.sems.allocated().values()))
        nc.all_engine_barrier()

    if getattr(tile.TileContext, "_gcn_patched", False):
        return
    tile.TileContext._drain_and_barrier = _drain_and_barrier_split
    tile.TileContext._gcn_patched = True

    ctr = [0]

    def _split_bir_waits(bir_json: bytes) -> bytes:
        d = _json.loads(bir_json)
        changed = False
        for fn in d.get("functions", []):
            for blk in fn.get("blocks", []):
                new_insts = []
                for ins in blk.get("instructions", []):
                    si = ins.get("sync_info")
                    waits = (si or {}).get("on_wait") or []
                    if len(waits) > 1:
                        for w in waits[1:]:
                            ctr[0] += 1
                            new_insts.append({
                                "debug": ins.get("debug", 0),
                                "engine": ins["engine"],
                                "ins": [], "outs": [],
                                "name": f"waitsplit-{ctr[0]}",
                                "opcode": "EventSemaphore",
                                "sync_info": {"on_update": [],
                                              "on_wait": [w]},
                            })
                        si["on_wait"] = waits[:1]
                        changed = True
                    new_insts.append(ins)
                blk["instructions"] = new_insts
        return _json.dumps(d).encode() if changed else bir_json

    import concourse.bass_utils as bass_utils
    import concourse.bass2jax as bass2jax

    orig_compile = bass_utils.compile_bir_kernel

    def compile_split(bir_json, tmpdir, neff_name="file.neff"):
        return orig_compile(_split_bir_waits(bir_json), tmpdir, neff_name)

    bass_utils.compile_bir_kernel = compile_split
    bass2jax.compile_bir_kernel = compile_split
    bass_utils.upload_artifacts = lambda tmpdir: f"file://{tmpdir}"

    import sys
    if "antenv.axon_hooks" not in sys.modules:
        mod = types.ModuleType("antenv.axon_hooks")
        hook = [None]
        mod.set_axon_ntff_profile_hook = lambda h: hook.__setitem__(0, h)
        mod.get_axon_ntff_profile_hook = lambda: hook[0]
        sys.modules["antenv.axon_hooks"] = mod
        try:
            import antenv
            antenv.axon_hooks = mod
        except ImportError:
            pass
        try:
            from trn_agent_boot.trn_boot import _ntff_profile_via_ctypes
            so = "/opt/axon/libaxon_pjrt.so"
            if os.path.exists(so):
                mod.set_axon_ntff_profile_hook(_ntff_profile_via_ctypes(so))
        except Exception:
            pass


# ----------------------------------------------------------------------------
# host-side graph preprocessing
# ----------------------------------------------------------------------------
def _grid_and_chunks(cap):
    """Given capacity vector [NSEG] (desc order), return slot offsets and
    chunk/run structure.  Chunks are CHUNK slots, cut at segment boundaries
    (pad to the grid)."""
    slot0 = np.zeros(NSEG, np.int64)
    chunks = []       # (slot_lo, slot_hi, runs=[(cap, rank0, n, off_in_chunk)])
    pos = 0
    r = 0
    while r < NSEG and cap[r] > 0:
        lo = pos
        runs = []
        # fill one chunk
        while r < NSEG and cap[r] > 0 and (pos - lo) + cap[r] <= CHUNK:
            c = cap[r]
            r0 = r
            off = pos - lo
            while (r < NSEG and cap[r] == c
                   and (pos - lo) + c <= CHUNK):
                slot0[r] = pos
                pos += c
                r += 1
            runs.append((int(c), int(r0), int(r - r0), int(off)))
        # pad chunk to CHUNK
        pos = lo + CHUNK
        chunks.append((int(lo), int(pos), runs))
    total = pos
    return slot0, chunks, int(total)


def _prep2(edge_index, n_nodes):
    src = np.asarray(edge_index[0], dtype=np.int64)
    dst = np.asarray(edge_index[1], dtype=np.int64)
    S = np.r_[src, np.arange(n_nodes)]
    D = np.r_[dst, np.arange(n_nodes)]
    deg = np.bincount(D, minlength=n_nodes).astype(np.float64)
    dinv = deg ** -0.5
    w_e = (dinv[S] * dinv[D]).astype(np.float32)

    core = D // NPC
    loc = D % NPC
    grp = np.searchsorted(_GB, loc, side="right") - 1
    seg = loc - _GB[grp]
    stream = (core * 8 + grp)                      # 64 streams

    # columns per layer for the SOURCE node
    c2 = S // NPC
    l2 = S % NPC
    g2 = np.searchsorted(_GB, l2, side="right") - 1
    s2 = l2 - _GB[g2]
    colL = [c2 * SHARD + l2, c2 * SHARD + g2 * GS + s2]

    layers = []
    for L in range(2):
        col = colL[L]
        win = col // WINN
        pairid = ((col % WINN) // 2).astype(np.int64)
        parity = (col % 2).astype(np.int64)

        # per (stream, win, seg) degree
        key = (stream * NW + win) * NSEG + seg
        cnt = np.bincount(key, minlength=64 * NW * NSEG).reshape(64, NW, NSEG)

        wins = []
        # rank of each seg within its (stream, win): stable sort by -deg
        for wdx in range(NW):
            cw = cnt[:, wdx, :]
            order = np.argsort(-cw, axis=1, kind="stable")
            rank = np.empty_like(order)
            np.put_along_axis(rank, order, np.arange(NSEG)[None, :].repeat(64, 0), axis=1)
            cap = np.take_along_axis(cw, order, axis=1).max(axis=0)
            slot0, chunks, total = _grid_and_chunks(cap)
            wins.append(dict(rank=rank, cap=cap, slot0=slot0,
                             chunks=chunks, total=total, cnt=cw))
        # edge slot positions
        # order edges by (stream, win, seg) then cumcount
        eord = np.lexsort((seg, win, stream))
        ss, ww, gg = stream[eord], win[eord], seg[eord]
        k2 = (ss * NW + ww) * NSEG + gg
        brk = np.r_[True, k2[1:] != k2[:-1]]
        gidx = np.cumsum(brk) - 1
        start_of = np.flatnonzero(brk)
        within = np.arange(len(k2)) - start_of[gidx]
        slot = np.empty(len(k2), np.int64)
        for wdx in range(NW):
            m = ww == wdx
            rk = wins[wdx]["rank"][ss[m], gg[m]]
            slot[m] = wins[wdx]["slot0"][rk] + within[m]
        layers.append(dict(win=win, pairid=pairid, parity=parity,
                           wins=wins, eord=eord, slot=slot,
                           ss=ss, ww=ww))

    return dict(S=S, D=D, dinv=dinv, w_e=w_e, core=core, grp=grp, seg=seg,
                stream=stream, layers=layers)


def _wrap_idx(stream_vals, ngrp_streams):
    """[8 groups][slots] int16 -> [128, slots//16] wrapped per group."""
    slots = stream_vals.shape[1]
    out = np.zeros((128, slots // 16), np.int16)
    for g in range(8):
        a = stream_vals[g].reshape(slots // 16, 16).T   # [16, cols]
        out[g * 16:(g + 1) * 16] = a
    return out


def _core_arrays(prep, c):
    """Build per-core device input arrays."""
    S, D = prep["S"], prep["D"]
    core, grp, seg = prep["core"], prep["grp"], prep["seg"]
    w_e = prep["w_e"]
    out = {}
    for L, lay in enumerate(prep["layers"]):
        eord, slot = lay["eord"], lay["slot"]
        ss, ww = lay["ss"], lay["ww"]
        pid = lay["pairid"][eord]
        par = lay["parity"][eord]
        wv = w_e[eord]
        m_core = (ss // 8) == c
        for wdx in range(NW):
            total = lay["wins"][wdx]["total"]
            idxs = np.zeros((8, total), np.int16)
            mh = np.zeros((8, total), np.float32)
            m = m_core & (ww == wdx)
            gsel = ss[m] % 8
            sl = slot[m]
            idxs[gsel, sl] = pid[m].astype(np.int16)
            mh[gsel, sl] = wv[m] * np.where(par[m] == 0, 1.0, -1.0)
            out[f"idx{L}_{wdx}"] = _wrap_idx(idxs, 8)
            mhf = np.zeros((128, total), _bf16)
            for g in range(8):
                mhf[g * 16:(g + 1) * 16] = mh[g][None, :].astype(_bf16)
            out[f"mh{L}_{wdx}"] = mhf
        # alignment stream: per group, interleaved [rank_w0 | 1568+rank_w1]
        al = np.full((8, 3136), 3135, np.int16)
        for g in range(8):
            nsg = int(_GB[g + 1] - _GB[g])
            r0 = lay["wins"][0]["rank"][c * 8 + g, :nsg]
            r1 = lay["wins"][1]["rank"][c * 8 + g, :nsg]
            al[g, 0:2 * nsg:2] = r0.astype(np.int16)
            al[g, 1:2 * nsg:2] = (1568 + r1).astype(np.int16)
        out[f"al{L}"] = _wrap_idx(al, 8)
    return out


# ----------------------------------------------------------------------------
# bass kernel builder
# ----------------------------------------------------------------------------
def _build2(plan):
    import concourse.bass as bass
    import concourse.bacc as bacc
    import concourse.mybir as mybir
    import concourse.tile as tile

    f32, bf16, i16 = mybir.dt.float32, mybir.dt.bfloat16, mybir.dt.int16
    AP = bass.AP
    nc = bacc.Bacc(None, target_bir_lowering=False)

    xT = nc.declare_dram_parameter("xT", [128, 2 * SHARD], bf16, isOutput=False)
    w1 = nc.declare_dram_parameter("w1", [128, 32], bf16, isOutput=False)
    wdsel = nc.declare_dram_parameter("wdsel", [128, 8], bf16, isOutput=False)
    b1v = nc.declare_dram_parameter("b1v", [128, 1], f32, isOutput=False)
    bdv = nc.declare_dram_parameter("bdv", [128, 1], f32, isOutput=False)
    idx_p, mh_p, al_p = {}, {}, {}
    for L in range(2):
        for wdx in range(NW):
            t = plan["totals"][L][wdx]
            idx_p[(L, wdx)] = nc.declare_dram_parameter(
                f"idx{L}_{wdx}", [128, t // 16], i16, isOutput=False)
            mh_p[(L, wdx)] = nc.declare_dram_parameter(
                f"mh{L}_{wdx}", [128, t], bf16, isOutput=False)
        al_p[L] = nc.declare_dram_parameter(f"al{L}", [128, 196], i16,
                                            isOutput=False)
    out_p = nc.declare_dram_parameter("out", [128, 3136], f32, isOutput=True)

    agin1 = nc.dram_tensor("agin1", [16, SHARD], bf16)
    table1 = nc.dram_tensor("table1", [128, SHARD], bf16, addr_space="Shared")
    agin2 = nc.dram_tensor("agin2", [1, SHARD], bf16)
    table2 = nc.dram_tensor("table2", [8, SHARD], bf16, addr_space="Shared")

    with tile.TileContext(nc) as tc:
        with tc.tile_pool(name="sb", bufs=1) as P1, \
             tc.tile_pool(name="dbl", bufs=2) as P2, \
             tc.tile_pool(name="ps", bufs=2, space="PSUM") as PP:

            # ---- constants ----
            w1_t = P1.tile([128, 32], bf16)
            nc.sync.dma_start(out=w1_t[:], in_=w1[:])
            wdsel_t = P1.tile([128, 8], bf16)
            nc.sync.dma_start(out=wdsel_t[:], in_=wdsel[:])
            b1_t = P1.tile([128, 1], f32)
            nc.sync.dma_start(out=b1_t[:], in_=b1v[:])
            bd_t = P1.tile([128, 1], f32)
            nc.sync.dma_start(out=bd_t[:], in_=bdv[:])

            # ---- layer-1 shard: gT = (x @ W1)^T  [16, SHARD] bf16 ----
            # xk shares the big slot with the window tables (disjoint life).
            xk = P1.tile([128, 2 * SHARD], bf16, tag="tblxk")
            nc.sync.dma_start(out=xk[:], in_=xT[:])
            gT = P1.tile([16, SHARD], bf16, tag="gTout")
            NCOL = 512
            for j0 in range(0, SHARD, NCOL):
                ncol = min(NCOL, SHARD - j0)
                ps = PP.tile([16, NCOL], f32, tag="mm")
                nc.tensor.matmul(out=ps[:, :ncol], lhsT=w1_t[:, 0:16],
                                 rhs=xk[:, j0:j0 + ncol],
                                 start=True, stop=False)
                nc.tensor.matmul(out=ps[:, :ncol], lhsT=w1_t[:, 16:32],
                                 rhs=xk[:, SHARD + j0:SHARD + j0 + ncol],
                                 start=False, stop=True)
                nc.vector.tensor_copy(out=gT[:, j0:j0 + ncol], in_=ps[:, :ncol])
            nc.sync.dma_start(out=agin1[:, :], in_=gT[:, :])
            nc.gpsimd.collective_compute(
                "AllGather", mybir.AluOpType.bypass,
                replica_groups=[list(range(NCORES))],
                ins=[agin1[:, :].opt()], outs=[table1[:, :].opt()])

            partials = P1.tile([128, 3200], f32, tag="partials")
            delta = P1.tile([128, 1568], f32, tag="delta")

            for L in range(2):
                nc.vector.memset(partials[:], 0.0)
                for wdx in range(NW):
                    # ---- load table window (8 per-group replicas) ----
                    tbl = P1.tile([128, WINN], bf16, tag="tblxk")
                    if L == 0:
                        for g in range(8):
                            nc.sync.dma_start(
                                out=tbl[g * 16:(g + 1) * 16, :],
                                in_=AP(table1[:, :].tensor,
                                       table1[:, :].offset
                                       + wdx * 4 * 16 * SHARD,
                                       [(SHARD, 16), (16 * SHARD, 4),
                                        (1, SHARD)]))
                    else:
                        for g in range(8):
                            nc.sync.dma_start(
                                out=tbl[g * 16:(g + 1) * 16, :],
                                in_=AP(table2[:, :].tensor,
                                       table2[:, :].offset + wdx * 4 * SHARD,
                                       [(0, 16), (SHARD, 4), (1, SHARD)]))
                    total = plan["totals"][L][wdx]
                    idxt = P1.tile([128, plan["maxtot"] // 16], i16,
                                   tag="idxthb")
                    nc.sync.dma_start(out=idxt[:, :total // 16],
                                      in_=idx_p[(L, wdx)][:])
                    tblv = tbl[:, :].rearrange("p (n d) -> p n d", d=2)
                    for (lo, hi, runs) in plan["chunks"][L][wdx]:
                        msg = P2.tile([128, CHUNK, 2], bf16, tag="msg")
                        nc.gpsimd.ap_gather(
                            out_ap=msg[:, :, :], in_ap=tblv,
                            idxs_ap=idxt[:, lo // 16:hi // 16],
                            channels=128, num_elems=WINN // 2, d=2,
                            num_idxs=CHUNK)
                        mh = P1.tile([128, CHUNK], bf16, tag="mh")
                        nc.sync.dma_start(out=mh[:],
                                          in_=mh_p[(L, wdx)][:, lo:hi])
                        mask2 = P1.tile([128, CHUNK, 2], bf16, tag="sc12")
                        nc.vector.tensor_scalar(
                            out=mask2[:, :, 0:1].rearrange("p n d -> p (n d)"),
                            in0=mh[:, :], scalar1=0.0, scalar2=None,
                            op0=mybir.AluOpType.max)
                        nc.vector.tensor_scalar(
                            out=mask2[:, :, 1:2].rearrange("p n d -> p (n d)"),
                            in0=mh[:, :], scalar1=-1.0, scalar2=0.0,
                            op0=mybir.AluOpType.mult,
                            op1=mybir.AluOpType.max)
                        nc.vector.tensor_mul(out=msg[:, :, :],
                                             in0=msg[:, :, :],
                                             in1=mask2[:, :, :])
                        poff = 0 if wdx == 0 else 1568
                        mv = msg[:, :, :].rearrange("p n d -> p (n d)")
                        for (cap, r0, n, off) in runs:
                            inv = AP(mv.tensor, mv.offset + off * 2,
                                     [mv.ap[0], (cap * 2, n), (1, cap * 2)])
                            nc.vector.tensor_reduce(
                                out=partials[:, poff + r0:poff + r0 + n],
                                in_=inv, axis=mybir.AxisListType.X,
                                op=mybir.AluOpType.add)

                # ---- alignment gather + combine windows ----
                alt = P1.tile([128, 196], i16, tag="altd8")
                nc.sync.dma_start(out=alt[:], in_=al_p[L][:])
                alo = P1.tile([128, 3136, 1], f32, tag="sc12")
                nc.gpsimd.ap_gather(
                    out_ap=alo[:, :, :],
                    in_ap=partials[:, :3136].unsqueeze(-1),
                    idxs_ap=alt[:, :], channels=128, num_elems=3136, d=1,
                    num_idxs=3136)
                q = delta
                nc.vector.tensor_reduce(
                    out=q[:, :],
                    in_=alo[:, :, :].rearrange("p (s two) d -> p s (two d)",
                                               two=2),
                    axis=mybir.AxisListType.X, op=mybir.AluOpType.add)

                if L == 0:
                    b1_b = b1_t[:, :].to_broadcast([128, 1568])
                    nc.vector.tensor_add(out=q[:, :], in0=q[:, :], in1=b1_b)
                    nc.vector.tensor_scalar(out=q[:, :], in0=q[:, :],
                                            scalar1=0.0, scalar2=None,
                                            op0=mybir.AluOpType.max)
                    hb = P1.tile([128, 1568], bf16, tag="idxthb")
                    nc.vector.tensor_copy(out=hb[:, :], in_=q[:, :])
                    # delta8 = wdsel^T @ h  [8, 1568]
                    d8 = P1.tile([8, 1568], bf16, tag="altd8")
                    for j0 in range(0, 1536, 512):
                        ps = PP.tile([8, 512], f32, tag="mm2")
                        nc.tensor.matmul(out=ps[:], lhsT=wdsel_t[:, :],
                                         rhs=hb[:, j0:j0 + 512],
                                         start=True, stop=True)
                        nc.vector.tensor_copy(out=d8[:, j0:j0 + 512],
                                              in_=ps[:])
                    ps = PP.tile([8, 32], f32, tag="mm2")
                    nc.tensor.matmul(out=ps[:], lhsT=wdsel_t[:, :],
                                     rhs=hb[:, 1536:1568],
                                     start=True, stop=True)
                    nc.vector.tensor_copy(out=d8[:, 1536:1568], in_=ps[:])
                    nc.sync.dma_start(
                        out=AP(agin2[:, :].tensor, agin2[:, :].offset,
                               [(1568, 8), (1, 1568)]),
                        in_=d8[:, :])
                    nc.gpsimd.collective_compute(
                        "AllGather", mybir.AluOpType.bypass,
                        replica_groups=[list(range(NCORES))],
                        ins=[agin2[:, :].opt()], outs=[table2[:, :].opt()])
                else:
                    bd_b = bd_t[:, :].to_broadcast([128, 1568])
                    nc.vector.tensor_add(out=q[:, :], in0=q[:, :], in1=bd_b)
                    # log_softmax pair from delta
                    m_t = P1.tile([128, 1568], f32, tag="partials")
                    nc.vector.tensor_scalar(out=m_t[:, :], in0=q[:, :],
                                            scalar1=0.0, scalar2=None,
                                            op0=mybir.AluOpType.max)
                    e1 = P1.tile([128, 1568], f32, tag="tblxk")
                    nc.vector.tensor_sub(out=e1[:, :], in0=q[:, :],
                                         in1=m_t[:, :])
                    nc.scalar.activation(e1[:, :], e1[:, :],
                                         mybir.ActivationFunctionType.Exp)
                    e2 = P1.tile([128, 1568], f32, tag="sc12")
                    nc.scalar.activation(e2[:, :], m_t[:, :],
                                         mybir.ActivationFunctionType.Exp,
                                         bias=0.0, scale=-1.0)
                    nc.vector.tensor_add(out=e1[:, :], in0=e1[:, :],
                                         in1=e2[:, :])
                    nc.scalar.activation(e1[:, :], e1[:, :],
                                         mybir.ActivationFunctionType.Ln)
                    nc.vector.tensor_add(out=e1[:, :], in0=e1[:, :],
                                         in1=m_t[:, :])   # e1 = softplus(q)
                    outt = P1.tile([128, 1568, 2], f32, tag="gTout")
                    nc.vector.tensor_scalar(
                        out=outt[:, :, 0:1].rearrange("p n d -> p (n d)"),
                        in0=e1[:, :], scalar1=-1.0, scalar2=None,
                        op0=mybir.AluOpType.mult)
                    nc.vector.tensor_sub(
                        out=outt[:, :, 1:2].rearrange("p n d -> p (n d)"),
                        in0=q[:, :], in1=e1[:, :])
                    nc.sync.dma_start(
                        out=out_p[:, :],
                        in_=outt[:, :, :].rearrange("p n d -> p (n d)"))

    nc.finalize()
    return nc


# ----------------------------------------------------------------------------
# public entry
# ----------------------------------------------------------------------------
_CACHE = {}


def kernel(x, edge_index, W1, b1, W2, b2):
    _install_patches()
    from concourse.bass_utils import run_bass_kernel_spmd

    n = x.shape[0]
    prep = _prep2(edge_index, n)

    totals = [[prep["layers"][L]["wins"][wdx]["total"] for wdx in range(NW)]
              for L in range(2)]
    chunks = [[prep["layers"][L]["wins"][wdx]["chunks"] for wdx in range(NW)]
              for L in range(2)]
    plan = dict(totals=totals, chunks=chunks,
                maxtot=max(max(t) for t in totals))

    key = tuple(tuple((lo, hi, tuple(runs)) for (lo, hi, runs) in chunks[L][wdx])
                for L in range(2) for wdx in range(NW))
    if key not in _CACHE:
        _CACHE.clear()
        _CACHE[key] = _build2(plan)
    nc = _CACHE[key]

    wdiff = (W2[:, 1] - W2[:, 0]).astype(np.float32)
    bdiff = np.float32(b2[1] - b2[0])

    w1p = np.ascontiguousarray(
        W1.astype(np.float32).reshape(2, 128, HID).transpose(1, 0, 2)
        .reshape(128, 32)).astype(_bf16)
    wdsel = np.zeros((128, 8), np.float32)
    for p in range(128):
        wdsel[p, p // 16] = wdiff[p % 16]
    wdsel = wdsel.astype(_bf16)
    b1p = np.asarray(b1, np.float32)[np.tile(np.arange(HID), 8)].reshape(128, 1)

    in_maps = []
    for c in range(NCORES):
        xc = np.zeros((SHARD, N_FEAT), np.float32)
        xc[:NPC] = x[c * NPC:(c + 1) * NPC]
        xTc = np.ascontiguousarray(
            xc.T.reshape(2, 128, SHARD).transpose(1, 0, 2)
            .reshape(128, 2 * SHARD)).astype(_bf16)
        m = dict(xT=xTc, w1=w1p, wdsel=wdsel,
                 b1v=np.ascontiguousarray(b1p),
                 bdv=np.full((128, 1), bdiff, np.float32))
        m.update(_core_arrays(prep, c))
        in_maps.append(m)

    trace = bool(int(os.environ.get("GCN_TRACE", "0")))
    res = run_bass_kernel_spmd(nc, in_maps, core_ids=list(range(NCORES)),
                               trace=trace)
    _EXEC_TIME_NS[0] = res.exec_time_ns
    _LAST_RES[0] = res

    out = np.empty((n, 2), np.float32)
    for c in range(NCORES):
        o = res.results[c]["out"]
        for g in range(8):
            nsg = int(_GB[g + 1] - _GB[g])
            row = o[16 * g].reshape(1568, 2)
            out[c * NPC + _GB[g]:c * NPC + _GB[g + 1]] = row[:nsg]
    return out


# revision 3
# speedup vs baseline: 3.0804x; 1.2120x over previous
"""GCN (2-layer GCNConv + log_softmax) on 8 Trainium2 NeuronCores — v2.

Architecture (per core, per layer):
  - Feature-transposed tables: [16 feat, nodes] bf16, replicated per Q7
    core group.  Aggregation via gpsimd ap_gather (8 concurrent per-group
    edge streams, one index per edge, d=2 node-pairs), DVE mask-multiply
    (mask carries the full sym-norm and parity selection), DVE segment
    reduce over a rank-capacity grid shared by all 64 (core,group)
    streams, gpsimd ap_gather alignment across the 2 table windows.
  - Edges partitioned by dst into 8 groups per core (one per Q7 core);
    layer matmuls on PE produce transposed shards directly; AllGather
    exchanges transposed shards; W2 delta folded into the layer-2 table.
"""

import os
import numpy as np
import ml_dtypes

NCORES = 8
NPC = 12500            # nodes per core
SHARD = 12544          # padded shard width (= 98*128)
GS = 1568              # group stride in layer-2 column space
NSEG = 1563            # max segments per group (4 groups of 1563, 4 of 1562)
WINN = 50176           # nodes per window (4 shards)
NW = 2                 # windows
CHUNK = 3840           # ap_gather slots per chunk
N_FEAT = 256
HID = 16

_bf16 = ml_dtypes.bfloat16
_EXEC_TIME_NS = [None]
_LAST_RES = [None]

_GB = np.r_[0, np.cumsum([1563] * 4 + [1562] * 4)]   # group row boundaries


def _install_patches():
    import json as _json
    import types
    import concourse.tile as tile
    import concourse.mybir as mybir
    from concourse.vector_clock import ScopedClock

    def _drain_and_barrier_split(self, tick_clock, wait_clock):
        nc = self.nc
        anchor = nc.sync.nop(hint="drain_wait_anchor", nofuse=True)
        wait_clock.add_sem_waits(
            anchor.ins, ScopedClock({None: tick_clock.global_clock})
        )
        waits = list(anchor.ins.sync_info.on_wait)
        anchor.ins.sync_info.on_wait = waits[:1]
        for w in waits[1:]:
            nop_inst = nc.sync.nop(hint="drain_wait_split", nofuse=True)
            nop_inst.ins.sync_info = mybir.SyncInfo(on_wait=[w], on_update=[])
        nc.sync.drain()
        nc.all_engine_barrier()
        assert self.sems is not None
        popped = nc._tile_sem_poison_stack.pop()
        assert popped is self._sem_poison
        nc.clear_and_free_semaphores(list(self.sems.allocated().values()))
        nc.all_engine_barrier()

    if getattr(tile.TileContext, "_gcn_patched", False):
        return
    tile.TileContext._drain_and_barrier = _drain_and_barrier_split
    tile.TileContext._gcn_patched = True

    ctr = [0]

    def _split_bir_waits(bir_json: bytes) -> bytes:
        d = _json.loads(bir_json)
        changed = False
        for fn in d.get("functions", []):
            for blk in fn.get("blocks", []):
                new_insts = []
                for ins in blk.get("instructions", []):
                    si = ins.get("sync_info")
                    waits = (si or {}).get("on_wait") or []
                    if len(waits) > 1:
                        for w in waits[1:]:
                            ctr[0] += 1
                            new_insts.append({
                                "debug": ins.get("debug", 0),
                                "engine": ins["engine"],
                                "ins": [], "outs": [],
                                "name": f"waitsplit-{ctr[0]}",
                                "opcode": "EventSemaphore",
                                "sync_info": {"on_update": [],
                                              "on_wait": [w]},
                            })
                        si["on_wait"] = waits[:1]
                        changed = True
                    new_insts.append(ins)
                blk["instructions"] = new_insts
        return _json.dumps(d).encode() if changed else bir_json

    import concourse.bass_utils as bass_utils
    import concourse.bass2jax as bass2jax

    orig_compile = bass_utils.compile_bir_kernel

    def compile_split(bir_json, tmpdir, neff_name="file.neff"):
        return orig_compile(_split_bir_waits(bir_json), tmpdir, neff_name)

    bass_utils.compile_bir_kernel = compile_split
    bass2jax.compile_bir_kernel = compile_split
    bass_utils.upload_artifacts = lambda tmpdir: f"file://{tmpdir}"

    import sys
    if "antenv.axon_hooks" not in sys.modules:
        mod = types.ModuleType("antenv.axon_hooks")
        hook = [None]
        mod.set_axon_ntff_profile_hook = lambda h: hook.__setitem__(0, h)
        mod.get_axon_ntff_profile_hook = lambda: hook[0]
        sys.modules["antenv.axon_hooks"] = mod
        try:
            import antenv
            antenv.axon_hooks = mod
        except ImportError:
            pass
        try:
            from trn_agent_boot.trn_boot import _ntff_profile_via_ctypes
            so = "/opt/axon/libaxon_pjrt.so"
            if os.path.exists(so):
                mod.set_axon_ntff_profile_hook(_ntff_profile_via_ctypes(so))
        except Exception:
            pass


# ----------------------------------------------------------------------------
# host-side graph preprocessing
# ----------------------------------------------------------------------------
def _grid_and_chunks(cap):
    """Given capacity vector [NSEG] (desc order), return slot offsets and
    chunk/run structure.  Chunks are CHUNK slots, cut at segment boundaries
    (pad to the grid)."""
    slot0 = np.zeros(NSEG, np.int64)
    chunks = []       # (slot_lo, slot_hi, runs=[(cap, rank0, n, off_in_chunk)])
    pos = 0
    r = 0
    while r < NSEG and cap[r] > 0:
        lo = pos
        runs = []
        # fill one chunk
        while r < NSEG and cap[r] > 0 and (pos - lo) + cap[r] <= CHUNK:
            c = cap[r]
            r0 = r
            off = pos - lo
            while (r < NSEG and cap[r] == c
                   and (pos - lo) + c <= CHUNK):
                slot0[r] = pos
                pos += c
                r += 1
            runs.append((int(c), int(r0), int(r - r0), int(off)))
        if r < NSEG and cap[r] > 0:
            pos = lo + CHUNK          # full interior chunk
        else:
            pos = lo + ((pos - lo + 15) // 16) * 16   # trim final chunk
        chunks.append((int(lo), int(pos), runs))
    total = pos
    return slot0, chunks, int(total)


def _prep2(edge_index, n_nodes):
    src = np.asarray(edge_index[0], dtype=np.int64)
    dst = np.asarray(edge_index[1], dtype=np.int64)
    S = np.r_[src, np.arange(n_nodes)]
    D = np.r_[dst, np.arange(n_nodes)]
    deg = np.bincount(D, minlength=n_nodes).astype(np.float64)
    dinv = deg ** -0.5
    w_e = (dinv[S] * dinv[D]).astype(np.float32)

    core = D // NPC
    loc = D % NPC
    grp = np.searchsorted(_GB, loc, side="right") - 1
    seg = loc - _GB[grp]
    stream = (core * 8 + grp)                      # 64 streams

    # columns per layer for the SOURCE node
    c2 = S // NPC
    l2 = S % NPC
    g2 = np.searchsorted(_GB, l2, side="right") - 1
    s2 = l2 - _GB[g2]
    colL = [c2 * SHARD + l2, c2 * SHARD + g2 * GS + s2]

    layers = []
    for L in range(2):
        col = colL[L]
        win = col // WINN
        pairid = ((col % WINN) // 2).astype(np.int64)
        parity = (col % 2).astype(np.int64)

        # per (stream, win, seg) degree
        key = (stream * NW + win) * NSEG + seg
        cnt = np.bincount(key, minlength=64 * NW * NSEG).reshape(64, NW, NSEG)

        wins = []
        # rank of each seg within its (stream, win): stable sort by -deg
        for wdx in range(NW):
            cw = cnt[:, wdx, :]
            order = np.argsort(-cw, axis=1, kind="stable")
            rank = np.empty_like(order)
            np.put_along_axis(rank, order, np.arange(NSEG)[None, :].repeat(64, 0), axis=1)
            cap = np.take_along_axis(cw, order, axis=1).max(axis=0)
            slot0, chunks, total = _grid_and_chunks(cap)
            wins.append(dict(rank=rank, cap=cap, slot0=slot0, order=order,
                             chunks=chunks, total=total, cnt=cw))
        # edge slot positions
        # order edges by (stream, win, seg, pairid) then cumcount
        eord = np.lexsort((pairid, seg, win, stream))
        ss, ww, gg = stream[eord], win[eord], seg[eord]
        k2 = (ss * NW + ww) * NSEG + gg
        brk = np.r_[True, k2[1:] != k2[:-1]]
        gidx = np.cumsum(brk) - 1
        start_of = np.flatnonzero(brk)
        within = np.arange(len(k2)) - start_of[gidx]
        slot = np.empty(len(k2), np.int64)
        for wdx in range(NW):
            m = ww == wdx
            rk = wins[wdx]["rank"][ss[m], gg[m]]
            slot[m] = wins[wdx]["slot0"][rk] + within[m]
        layers.append(dict(win=win, pairid=pairid, parity=parity,
                           wins=wins, eord=eord, slot=slot,
                           ss=ss, ww=ww))

    return dict(S=S, D=D, dinv=dinv, w_e=w_e, core=core, grp=grp, seg=seg,
                stream=stream, layers=layers)


def _wrap_idx(stream_vals, ngrp_streams):
    """[8 groups][slots] int16 -> [128, slots//16] wrapped per group."""
    slots = stream_vals.shape[1]
    out = np.zeros((128, slots // 16), np.int16)
    for g in range(8):
        a = stream_vals[g].reshape(slots // 16, 16).T   # [16, cols]
        out[g * 16:(g + 1) * 16] = a
    return out


def _core_arrays(prep, c):
    """Build per-core device input arrays."""
    S, D = prep["S"], prep["D"]
    core, grp, seg = prep["core"], prep["grp"], prep["seg"]
    w_e = prep["w_e"]
    out = {}
    for L, lay in enumerate(prep["layers"]):
        eord, slot = lay["eord"], lay["slot"]
        ss, ww = lay["ss"], lay["ww"]
        pid = lay["pairid"][eord]
        par = lay["parity"][eord]
        wv = w_e[eord]
        m_core = (ss // 8) == c
        for wdx in range(NW):
            total = lay["wins"][wdx]["total"]
            idxs = np.zeros((8, total), np.int16)
            mh = np.zeros((8, total), np.float32)
            m = m_core & (ww == wdx)
            gsel = ss[m] % 8
            sl = slot[m]
            idxs[gsel, sl] = pid[m].astype(np.int16)
            mh[gsel, sl] = wv[m] * np.where(par[m] == 0, 1.0, -1.0)
            out[f"idx{L}_{wdx}"] = _wrap_idx(idxs, 8)
            mhf = np.zeros((128, total), _bf16)
            for g in range(8):
                mhf[g * 16:(g + 1) * 16] = mh[g][None, :].astype(_bf16)
            out[f"mh{L}_{wdx}"] = mhf
        # local_scatter alignment idx: dst position of the rank-i partial
        ls = np.full((8, 2 * 1564), -1, np.int16)
        for g in range(8):
            for wdx in range(NW):
                o = lay["wins"][wdx]["order"][c * 8 + g]
                ls[g, wdx * 1564:wdx * 1564 + NSEG] = o.astype(np.int16)
        out[f"ls{L}"] = np.repeat(ls, 16, axis=0)
    return out


# ----------------------------------------------------------------------------
# bass kernel builder
# ----------------------------------------------------------------------------
def _build2(plan):
    import concourse.bass as bass
    import concourse.bacc as bacc
    import concourse.mybir as mybir
    import concourse.tile as tile

    f32, bf16, i16 = mybir.dt.float32, mybir.dt.bfloat16, mybir.dt.int16
    AP = bass.AP
    nc = bacc.Bacc(None, target_bir_lowering=False)

    xT = nc.declare_dram_parameter("xT", [128, 2 * SHARD], bf16, isOutput=False)
    w1 = nc.declare_dram_parameter("w1", [128, 256], bf16, isOutput=False)
    wdsel = nc.declare_dram_parameter("wdsel", [128, 8], bf16, isOutput=False)
    b1v = nc.declare_dram_parameter("b1v", [128, 1], f32, isOutput=False)
    bdv = nc.declare_dram_parameter("bdv", [128, 1], f32, isOutput=False)
    idx_p, mh_p, al_p = {}, {}, {}
    for L in range(2):
        for wdx in range(NW):
            t = plan["totals"][L][wdx]
            idx_p[(L, wdx)] = nc.declare_dram_parameter(
                f"idx{L}_{wdx}", [128, t // 16], i16, isOutput=False)
            mh_p[(L, wdx)] = nc.declare_dram_parameter(
                f"mh{L}_{wdx}", [128, t], bf16, isOutput=False)
        al_p[L] = nc.declare_dram_parameter(f"ls{L}", [128, 3128], i16,
                                            isOutput=False)
    out_p = nc.declare_dram_parameter("out", [128, 3136], f32, isOutput=True)

    agin1 = nc.dram_tensor("agin1", [128, SHARD], bf16)
    table1 = nc.dram_tensor("table1", [1024, SHARD], bf16, addr_space="Shared")
    agin2 = nc.dram_tensor("agin2", [1, SHARD], bf16)
    table2 = nc.dram_tensor("table2", [8, SHARD], bf16, addr_space="Shared")

    with tile.TileContext(nc) as tc:
        with tc.tile_pool(name="sb", bufs=1) as P1, \
             tc.tile_pool(name="dbl", bufs=2) as P2, \
             tc.tile_pool(name="ps", bufs=2, space="PSUM") as PP:

            # ---- constants ----
            w1_t = P1.tile([128, 256], bf16)
            nc.sync.dma_start(out=w1_t[:], in_=w1[:])
            wdsel_t = P1.tile([128, 8], bf16)
            nc.sync.dma_start(out=wdsel_t[:], in_=wdsel[:])
            b1_t = P1.tile([128, 1], f32)
            nc.sync.dma_start(out=b1_t[:], in_=b1v[:])
            bd_t = P1.tile([128, 1], f32)
            nc.sync.dma_start(out=bd_t[:], in_=bdv[:])

            # ---- layer-1 shard: gT = (x @ W1)^T  [16, SHARD] bf16 ----
            # xk shares the big slot with the window tables (disjoint life).
            xk = P1.tile([128, 2 * SHARD], bf16, tag="tblxk")
            nc.sync.dma_start(out=xk[:], in_=xT[:])
            NCOL = 512
            for j0 in range(0, SHARD, NCOL):
                ncol = min(NCOL, SHARD - j0)
                ps = PP.tile([128, NCOL], f32, tag="mm")
                nc.tensor.matmul(out=ps[:, :ncol], lhsT=w1_t[:, 0:128],
                                 rhs=xk[:, j0:j0 + ncol],
                                 start=True, stop=False)
                nc.tensor.matmul(out=ps[:, :ncol], lhsT=w1_t[:, 128:256],
                                 rhs=xk[:, SHARD + j0:SHARD + j0 + ncol],
                                 start=False, stop=True)
                gc = P2.tile([128, NCOL], bf16, tag="gc")
                nc.vector.tensor_copy(out=gc[:, :ncol], in_=ps[:, :ncol])
                nc.sync.dma_start(out=agin1[:, j0:j0 + ncol],
                                  in_=gc[:, :ncol])
            nc.gpsimd.collective_compute(
                "AllGather", mybir.AluOpType.bypass,
                replica_groups=[list(range(NCORES))],
                ins=[agin1[:, :].opt()], outs=[table1[:, :].opt()])

            partials = P1.tile([128, 3200], f32, tag="partials")
            delta = P1.tile([128, 1568], f32, tag="delta")

            for L in range(2):
                nc.vector.memset(partials[:], 0.0)
                for wdx in range(NW):
                    # ---- load table window (8 per-group replicas) ----
                    tbl = P1.tile([128, WINN], bf16, tag="tblxk")
                    if L == 0:
                        nc.sync.dma_start(
                            out=tbl[:, :],
                            in_=AP(table1[:, :].tensor,
                                   table1[:, :].offset
                                   + wdx * 4 * 128 * SHARD,
                                   [(SHARD, 128), (128 * SHARD, 4),
                                    (1, SHARD)]))
                    else:
                        nc.sync.dma_start(
                            out=tbl[:, :],
                            in_=AP(table2[:, :].tensor,
                                   table2[:, :].offset + wdx * 4 * SHARD,
                                   [(0, 128), (SHARD, 4), (1, SHARD)]))
                    total = plan["totals"][L][wdx]
                    idxt = P2.tile([128, plan["maxtot"] // 16], i16,
                                   tag="idxt")
                    nc.sync.dma_start(out=idxt[:, :total // 16],
                                      in_=idx_p[(L, wdx)][:])
                    tblv = tbl[:, :].rearrange("p (n d) -> p n d", d=2)
                    for (lo, hi, runs) in plan["chunks"][L][wdx]:
                        nidx = hi - lo
                        msg = P2.tile([128, CHUNK, 2], bf16, tag="msg")
                        nc.gpsimd.ap_gather(
                            out_ap=msg[:, :nidx, :], in_ap=tblv,
                            idxs_ap=idxt[:, lo // 16:hi // 16],
                            channels=128, num_elems=WINN // 2, d=2,
                            num_idxs=nidx)
                        mh = P1.tile([128, CHUNK], bf16, tag="mhpb")
                        nc.sync.dma_start(out=mh[:, :nidx],
                                          in_=mh_p[(L, wdx)][:, lo:hi])
                        mask2 = P1.tile([128, CHUNK, 2], bf16, tag="sc12")
                        nc.vector.tensor_scalar(
                            out=mask2[:, :nidx, 0:1]
                            .rearrange("p n d -> p (n d)"),
                            in0=mh[:, :nidx], scalar1=0.0, scalar2=None,
                            op0=mybir.AluOpType.max)
                        nc.vector.tensor_scalar(
                            out=mask2[:, :nidx, 1:2]
                            .rearrange("p n d -> p (n d)"),
                            in0=mh[:, :nidx], scalar1=-1.0, scalar2=0.0,
                            op0=mybir.AluOpType.mult,
                            op1=mybir.AluOpType.max)
                        nc.vector.tensor_mul(out=msg[:, :nidx, :],
                                             in0=msg[:, :nidx, :],
                                             in1=mask2[:, :nidx, :])
                        poff = 0 if wdx == 0 else 1568
                        mv = msg[:, :, :].rearrange("p n d -> p (n d)")
                        for (cap, r0, n, off) in runs:
                            inv = AP(mv.tensor, mv.offset + off * 2,
                                     [mv.ap[0], (cap * 2, n), (1, cap * 2)])
                            nc.vector.tensor_reduce(
                                out=partials[:, poff + r0:poff + r0 + n],
                                in_=inv, axis=mybir.AxisListType.X,
                                op=mybir.AluOpType.add)

                # ---- alignment scatter + combine windows ----
                pb = P1.tile([128, 3200], bf16, tag="mhpb")
                nc.vector.tensor_copy(out=pb[:, :], in_=partials[:, :])
                lsi = P1.tile([128, 3128], i16, tag="lsid8")
                nc.sync.dma_start(out=lsi[:], in_=al_p[L][:])
                A0 = P1.tile([128, 1568], bf16, tag="A0")
                nc.gpsimd.local_scatter(
                    out_ap=A0[:, :], data_ap=pb[:, 0:1564],
                    idxs_ap=lsi[:, 0:1564], channels=128,
                    num_elems=1568, num_idxs=1564)
                A1 = P1.tile([128, 1568], bf16, tag="A1")
                nc.gpsimd.local_scatter(
                    out_ap=A1[:, :], data_ap=pb[:, 1568:3132],
                    idxs_ap=lsi[:, 1564:3128], channels=128,
                    num_elems=1568, num_idxs=1564)
                q = delta
                nc.vector.tensor_add(out=q[:, :], in0=A0[:, :], in1=A1[:, :])

                if L == 0:
                    b1_b = b1_t[:, :].to_broadcast([128, 1568])
                    nc.vector.tensor_add(out=q[:, :], in0=q[:, :], in1=b1_b)
                    nc.vector.tensor_scalar(out=q[:, :], in0=q[:, :],
                                            scalar1=0.0, scalar2=None,
                                            op0=mybir.AluOpType.max)
                    hb = P1.tile([128, 1568], bf16, tag="hb")
                    nc.vector.tensor_copy(out=hb[:, :], in_=q[:, :])
                    # delta8 = wdsel^T @ h  [8, 1568]
                    d8 = P1.tile([8, 1568], bf16, tag="lsid8")
                    for j0 in range(0, 1536, 512):
                        ps = PP.tile([8, 512], f32, tag="mm2")
                        nc.tensor.matmul(out=ps[:], lhsT=wdsel_t[:, :],
                                         rhs=hb[:, j0:j0 + 512],
                                         start=True, stop=True)
                        nc.vector.tensor_copy(out=d8[:, j0:j0 + 512],
                                              in_=ps[:])
                    ps = PP.tile([8, 32], f32, tag="mm2")
                    nc.tensor.matmul(out=ps[:], lhsT=wdsel_t[:, :],
                                     rhs=hb[:, 1536:1568],
                                     start=True, stop=True)
                    nc.vector.tensor_copy(out=d8[:, 1536:1568], in_=ps[:])
                    nc.sync.dma_start(
                        out=AP(agin2[:, :].tensor, agin2[:, :].offset,
                               [(1568, 8), (1, 1568)]),
                        in_=d8[:, :])
                    nc.gpsimd.collective_compute(
                        "AllGather", mybir.AluOpType.bypass,
                        replica_groups=[list(range(NCORES))],
                        ins=[agin2[:, :].opt()], outs=[table2[:, :].opt()])
                else:
                    bd_b = bd_t[:, :].to_broadcast([128, 1568])
                    nc.vector.tensor_add(out=q[:, :], in0=q[:, :], in1=bd_b)
                    # log_softmax pair from delta
                    m_t = P1.tile([128, 1568], f32, tag="partials")
                    nc.vector.tensor_scalar(out=m_t[:, :], in0=q[:, :],
                                            scalar1=0.0, scalar2=None,
                                            op0=mybir.AluOpType.max)
                    e1 = P1.tile([128, 1568], f32, tag="tblxk")
                    nc.vector.tensor_sub(out=e1[:, :], in0=q[:, :],
                                         in1=m_t[:, :])
                    nc.scalar.activation(e1[:, :], e1[:, :],
                                         mybir.ActivationFunctionType.Exp)
                    e2 = P1.tile([128, 1568], f32, tag="sc12")
                    nc.scalar.activation(e2[:, :], m_t[:, :],
                                         mybir.ActivationFunctionType.Exp,
                                         bias=0.0, scale=-1.0)
                    nc.vector.tensor_add(out=e1[:, :], in0=e1[:, :],
                                         in1=e2[:, :])
                    nc.scalar.activation(e1[:, :], e1[:, :],
                                         mybir.ActivationFunctionType.Ln)
                    nc.vector.tensor_add(out=e1[:, :], in0=e1[:, :],
                                         in1=m_t[:, :])   # e1 = softplus(q)
                    outt = P1.tile([128, 1568, 2], f32, tag="partials")
                    nc.vector.tensor_scalar(
                        out=outt[:, :, 0:1].rearrange("p n d -> p (n d)"),
                        in0=e1[:, :], scalar1=-1.0, scalar2=None,
                        op0=mybir.AluOpType.mult)
                    nc.vector.tensor_sub(
                        out=outt[:, :, 1:2].rearrange("p n d -> p (n d)"),
                        in0=q[:, :], in1=e1[:, :])
                    nc.sync.dma_start(
                        out=out_p[:, :],
                        in_=outt[:, :, :].rearrange("p n d -> p (n d)"))

    nc.finalize()
    return nc


# ----------------------------------------------------------------------------
# public entry
# ----------------------------------------------------------------------------
_CACHE = {}


def kernel(x, edge_index, W1, b1, W2, b2):
    _install_patches()
    from concourse.bass_utils import run_bass_kernel_spmd

    n = x.shape[0]
    prep = _prep2(edge_index, n)

    totals = [[prep["layers"][L]["wins"][wdx]["total"] for wdx in range(NW)]
              for L in range(2)]
    chunks = [[prep["layers"][L]["wins"][wdx]["chunks"] for wdx in range(NW)]
              for L in range(2)]
    plan = dict(totals=totals, chunks=chunks,
                maxtot=max(max(t) for t in totals))

    key = tuple(tuple((lo, hi, tuple(runs)) for (lo, hi, runs) in chunks[L][wdx])
                for L in range(2) for wdx in range(NW))
    if key not in _CACHE:
        _CACHE.clear()
        _CACHE[key] = _build2(plan)
    nc = _CACHE[key]

    wdiff = (W2[:, 1] - W2[:, 0]).astype(np.float32)
    bdiff = np.float32(b2[1] - b2[0])

    w1p = np.zeros((128, 256), np.float32)
    for kc in range(2):
        for p in range(128):
            for m16 in range(8):
                w1p[p, kc * 128 + m16 * 16:kc * 128 + (m16 + 1) * 16] = \
                    W1[kc * 128 + p, :]
    w1p = np.ascontiguousarray(w1p).astype(_bf16)
    wdsel = np.zeros((128, 8), np.float32)
    for p in range(128):
        wdsel[p, p // 16] = wdiff[p % 16]
    wdsel = wdsel.astype(_bf16)
    b1p = np.asarray(b1, np.float32)[np.tile(np.arange(HID), 8)].reshape(128, 1)

    in_maps = []
    for c in range(NCORES):
        xc = np.zeros((SHARD, N_FEAT), np.float32)
        xc[:NPC] = x[c * NPC:(c + 1) * NPC]
        xTc = np.ascontiguousarray(
            xc.T.reshape(2, 128, SHARD).transpose(1, 0, 2)
            .reshape(128, 2 * SHARD)).astype(_bf16)
        m = dict(xT=xTc, w1=w1p, wdsel=wdsel,
                 b1v=np.ascontiguousarray(b1p),
                 bdv=np.full((128, 1), bdiff, np.float32))
        m.update(_core_arrays(prep, c))
        in_maps.append(m)

    trace = bool(int(os.environ.get("GCN_TRACE", "0")))
    res = run_bass_kernel_spmd(nc, in_maps, core_ids=list(range(NCORES)),
                               trace=trace)
    _EXEC_TIME_NS[0] = res.exec_time_ns
    _LAST_RES[0] = res

    out = np.empty((n, 2), np.float32)
    for c in range(NCORES):
        o = res.results[c]["out"]
        for g in range(8):
            nsg = int(_GB[g + 1] - _GB[g])
            row = o[16 * g].reshape(1568, 2)
            out[c * NPC + _GB[g]:c * NPC + _GB[g + 1]] = row[:nsg]
    return out
